# revision 16
# baseline (speedup 1.0000x reference)
"""Trainium2 Bass kernel for LoRALayer: out = 2.0 * (x @ B) @ A.

x: [4, 4096, 4096] f32; A: [8, 4096] f32; B: [4096, 8] f32.
Sharding: data-parallel on the 16384 tokens across 8 cores (2048 each);
A/B replicated. Host-side prep (part of sharding): each core's x-shard is
shipped transposed (contraction dim on SBUF partitions) as a single bf16
stream; B and 2*A likewise bf16. Output leaves the device as bf16 and the
host upconverts to f32 during the gather. Total HBM traffic is 32 MiB/core
and the kernel is fabric-bound (~435 GB/s/core SBUF AXI, ~216 GB/s per
DGE queue); bf16 rounding of x dominates the ~7e-3 absmax-rel error
(gate 2e-2).

Per core (T=2048), per 256-token block, PE work is packed with
tile_position concurrency so the (HAM-cold) PE never gates the streams:
  mm1 (4x col-tiled): feature chunk 4r+j accumulates into ps_y[32j:32j+8]
      (independent PE column groups, separate xbus streams) -> 8 rounds of
      4 concurrent 128x8x256 matmuls.
  y   = sum of 4 strips, split into two 128-token halves at partition
      bases 0/32 (bf16), matching A2 replicated at bases 0/32.
  mm2 (2x row-tiled): per 512-col chunk of A2, two concurrent rank-8
      matmuls (row groups 0/1) -> two PSUM banks; ACT copies subtile 0,
      DVE copies subtile 1 (PSUM->SBUF bf16).
DMA queue balance (each DGE queue drives 8 SDMA engines, ~216 GB/s):
  sync   = in0 + in1-on-odd-blocks   (12.6 MB, input only, no coupling)
  gpsimd = in1-on-even-blocks + out1 (12.6 MB)
  scalar = out0 only, emitted right after its own ACT copies (8.4 MB,
           waits trivially satisfied -> no head-of-line stalls)
"""

import numpy as np

P = 128
F_IN = 4096
F_OUT = 4096
RANK = 8
N_CORES = 8
SCALING = 2.0
TBLK = 256             # token block: 2 subtiles of 128 tokens

_CACHE = {}


def _build_nc(T, F_in, F_out, R):
    """Build the single-core Bass program for a T-token shard."""
    from contextlib import ExitStack

    import concourse.mybir as mybir
    import concourse.tile as tile
    from concourse import bacc

    f32 = mybir.dt.float32
    bf16 = mybir.dt.bfloat16
    tblk = min(TBLK, T)
    CH = F_in // P          # feature chunks (32)
    NB = T // tblk          # token blocks (8)
    NSUB = tblk // P        # 128-token subtiles per block (2)
    NS = F_out // 512       # output column chunks (8)
    NDMA = 4                # input sub-DMAs per block (512KB granularity:
    CGRP = CH // NDMA       # finer grain shortens the first-tile latency)
    RB = 32                 # partition-base alignment for engine APs
    CT = 4                  # mm1 column-tiling width

    nc = bacc.Bacc("TRN2", target_bir_lowering=False, debug=False)

    xh_d = nc.dram_tensor(
        "xh", [NB, NDMA, P, CGRP * tblk], bf16, kind="ExternalInput"
    ).ap()
    bpk_d = nc.dram_tensor("Bpk", [P, CH * R], bf16, kind="ExternalInput").ap()
    a2_d = nc.dram_tensor("A2", [R, F_out], bf16, kind="ExternalInput").ap()
    out_d = nc.dram_tensor("out", [T, F_out], bf16, kind="ExternalOutput").ap()

    with tile.TileContext(nc) as tc, ExitStack() as ctx:
        cpool = ctx.enter_context(tc.tile_pool(name="const", bufs=1))
        xtpool = ctx.enter_context(tc.tile_pool(name="xt", bufs=2 * NDMA))
        ypool = ctx.enter_context(tc.tile_pool(name="yt", bufs=2))
        opool = ctx.enter_context(tc.tile_pool(name="osb", bufs=8))
        y_pp = ctx.enter_context(tc.tile_pool(name="y_ps", bufs=2, space="PSUM"))
        o_pp = ctx.enter_context(tc.tile_pool(name="o_ps", bufs=3, space="PSUM"))

        bpk_sb = cpool.tile([P, CH * R], bf16, tag="bpk_sb")
        nc.sync.dma_start(bpk_sb[:], bpk_d)
        # A2 replicated at partition bases 0 and 32 for the row-tiled mm2.
        a2_sb = cpool.tile([RB + R, F_out], bf16, tag="a2_sb")
        nc.sync.dma_start(a2_sb[:R, :], a2_d)
        nc.sync.dma_start(a2_sb[RB:RB + R, :], a2_d)

        state = {}

        for blk in range(NB + 1):
            xts = []
            if blk < NB:
                for s in range(NDMA):
                    x_sb = xtpool.tile([P, CGRP, tblk], bf16, tag="x_sb")
                    eng = nc.sync if (s < 2 or blk % 2 == 1) else nc.gpsimd
                    eng.dma_start(
                        x_sb[:].rearrange("p c t -> p (c t)"), xh_d[blk, s]
                    )
                    xts.append(x_sb)
                ps_y = y_pp.tile([3 * RB + R, 512], f32, tag="ps_y")

            # All of mm2(blk-1) is emitted BEFORE mm1(blk): mm2 depends only
            # on y, never on fresh input, so it must not sit behind
            # input-waiting mm1 matmuls in the in-order PE queue (that
            # head-of-line blocking delayed the output stream by ~15us).
            if blk > 0:
                y_pk, _ = state.pop(blk - 1)
                o_sb0 = opool.tile(
                    [P, F_out], bf16, tag="o_sb0", name=f"o_sb0_{blk}"
                )
                o_sb1 = opool.tile(
                    [P, F_out], bf16, tag="o_sb1", name=f"o_sb1_{blk}"
                )
                trow = (blk - 1) * tblk
                for n in range(NS):
                    cs = slice(n * 512, (n + 1) * 512)
                    o_ps0 = o_pp.tile([P, 512], f32, tag="o_ps0")
                    o_ps1 = o_pp.tile([P, 512], f32, tag="o_ps1")
                    nc.tensor.matmul(
                        o_ps0[:], y_pk[:R, :], a2_sb[:R, cs],
                        start=True, stop=True,
                    )
                    nc.tensor.matmul(
                        o_ps1[:], y_pk[RB:RB + R, :], a2_sb[RB:RB + R, cs],
                        start=True, stop=True,
                    )
                    nc.scalar.copy(o_sb0[:, cs], o_ps0[:])
                    nc.vector.tensor_copy(o_sb1[:, cs], o_ps1[:])
                    # Fire the output DMAs per 2048-col half so the output
                    # stream starts as early as possible.
                    if n % (NS // 2) == NS // 2 - 1:
                        hs = slice((n + 1 - NS // 2) * 512, (n + 1) * 512)
                        nc.scalar.dma_start(out_d[trow:trow + P, hs], o_sb0[:, hs])
                        nc.gpsimd.dma_start(
                            out_d[trow + P:trow + 2 * P, hs], o_sb1[:, hs]
                        )
            if blk < NB:
                for r in range(CH // CT):
                    for j in range(CT):
                        c = CT * r + j
                        nc.tensor.matmul(
                            ps_y[j * RB:j * RB + R, :tblk],
                            bpk_sb[:, c * R:(c + 1) * R],
                            xts[c // CGRP][:, c % CGRP, :],
                            start=(r == 0), stop=(r == CH // CT - 1),
                            tile_position=(0, j * RB),
                        )
            if blk < NB:
                # y = sum of the 4 col-tile strips; two 128-token halves at
                # partition bases 0/32 (the mm2 row-tile weight layout).
                # DVE reads at most one PSUM operand per op -> stage in SBUF.
                yt = ypool.tile([R, tblk], f32, tag="yt")
                nc.vector.tensor_copy(yt[:], ps_y[:R, :tblk])
                for j in range(1, CT - 1):
                    nc.vector.tensor_add(
                        yt[:], yt[:], ps_y[j * RB:j * RB + R, :tblk]
                    )
                j = CT - 1
                y_pk = ypool.tile([RB + R, P], bf16, tag="y_pk")
                nc.vector.tensor_add(
                    y_pk[:R, :], yt[:, :P], ps_y[j * RB:j * RB + R, :P]
                )
                nc.vector.tensor_add(
                    y_pk[RB:RB + R, :], yt[:, P:tblk],
                    ps_y[j * RB:j * RB + R, P:tblk]
                )
                state[blk] = (y_pk, [])

    nc.compile()
    return nc


def _pack_inputs(x2d, A, B, T_shard, F_in, R):
    """Shard x on tokens (transposed, bf16); replicate B/A2 packs."""
    import ml_dtypes

    bf16 = ml_dtypes.bfloat16
    CH = F_in // P

    Bb = B.astype(np.float32).astype(bf16)
    bpk = np.ascontiguousarray(
        Bb.reshape(CH, P, R).transpose(1, 0, 2).reshape(P, CH * R)
    )
    a2 = np.ascontiguousarray((SCALING * A.astype(np.float32)).astype(bf16))

    T = T_shard
    tblk = min(TBLK, T)
    NB = T // tblk
    NDMA = 4
    CGRP = CH // NDMA

    def pack(m):
        a = m.reshape(NDMA, CGRP, P, NB, tblk)
        a = a.transpose(3, 0, 2, 1, 4)
        return np.ascontiguousarray(a.reshape(NB, NDMA, P, CGRP * tblk))

    n_shards = x2d.shape[0] // T_shard
    in_maps = []
    for c in range(n_shards):
        xt = np.ascontiguousarray(x2d[c * T_shard:(c + 1) * T_shard].T)
        in_maps.append({"xh": pack(xt.astype(bf16)), "Bpk": bpk, "A2": a2})
    return in_maps


def kernel(x, A, B):
    from concourse.bass_utils import run_bass_kernel_spmd

    x = np.asarray(x, dtype=np.float32)
    A = np.asarray(A, dtype=np.float32)
    B = np.asarray(B, dtype=np.float32)
    orig_shape = x.shape
    x2d = x.reshape(-1, F_IN)
    T_shard = x2d.shape[0] // N_CORES

    key = (T_shard, F_IN, F_OUT, RANK)
    if key not in _CACHE:
        _CACHE[key] = _build_nc(T_shard, F_IN, F_OUT, RANK)
    nc = _CACHE[key]

    in_maps = _pack_inputs(x2d, A, B, T_shard, F_IN, RANK)
    res = run_bass_kernel_spmd(nc, in_maps, core_ids=list(range(N_CORES)))
    out = np.concatenate(
        [np.asarray(r["out"], dtype=np.float32) for r in res.results], axis=0
    )
    return out.reshape(*orig_shape[:-1], F_OUT)


# revision 20
# speedup vs baseline: 1.1178x; 1.1178x over previous
"""Trainium2 Bass kernel for LoRALayer: out = 2.0 * (x @ B) @ A.

x: [4, 4096, 4096] f32; A: [8, 4096] f32; B: [4096, 8] f32.
Sharding: data-parallel on the 16384 tokens across 8 cores (2048 each);
A/B replicated. Host-side prep (part of sharding): each core's x-shard is
shipped transposed (contraction dim on SBUF partitions) as a single bf16
stream; B and 2*A likewise bf16. Output leaves the device as bf16 and the
host upconverts to f32 during the gather. Total HBM traffic is 32 MiB/core
and the kernel is fabric-bound (~435 GB/s/core SBUF AXI, ~216 GB/s per
DGE queue); bf16 rounding of x dominates the ~7e-3 absmax-rel error
(gate 2e-2).

Per core (T=2048), per 256-token block, PE work is packed with
tile_position concurrency so the (HAM-cold) PE never gates the streams:
  mm1 (4x col-tiled): feature chunk 4r+j accumulates into ps_y[32j:32j+8]
      (independent PE column groups, separate xbus streams) -> 8 rounds of
      4 concurrent 128x8x256 matmuls.
  y   = sum of 4 strips, split into two 128-token halves at partition
      bases 0/32 (bf16), matching A2 replicated at bases 0/32.
  mm2 (2x row-tiled): per 512-col chunk of A2, two concurrent rank-8
      matmuls (row groups 0/1) -> two PSUM banks; ACT copies subtile 0,
      DVE copies subtile 1 (PSUM->SBUF bf16).
DMA queue balance (each DGE queue drives 8 SDMA engines, ~216 GB/s):
  sync   = in0 + in1-on-odd-blocks   (12.6 MB, input only, no coupling)
  gpsimd = in1-on-even-blocks + out1 (12.6 MB)
  scalar = out0 only, emitted right after its own ACT copies (8.4 MB,
           waits trivially satisfied -> no head-of-line stalls)
"""

import numpy as np

P = 128
F_IN = 4096
F_OUT = 4096
RANK = 8
N_CORES = 8
SCALING = 2.0
TBLK = 256             # token block: 2 subtiles of 128 tokens

_CACHE = {}


def _build_nc(T, F_in, F_out, R):
    """Build the single-core Bass program for a T-token shard."""
    from contextlib import ExitStack

    import concourse.mybir as mybir
    import concourse.tile as tile
    from concourse import bacc

    f32 = mybir.dt.float32
    bf16 = mybir.dt.bfloat16
    tblk = min(TBLK, T)
    CH = F_in // P          # feature chunks (32)
    NB = T // tblk          # token blocks (8)
    NSUB = tblk // P        # 128-token subtiles per block (2)
    NS = F_out // 512       # output column chunks (8)
    NDMA = 2                # input sub-DMAs per block (1MB granularity)
    CGRP = CH // NDMA
    RB = 32                 # partition-base alignment for engine APs
    CT = 4                  # mm1 column-tiling width

    nc = bacc.Bacc("TRN2", target_bir_lowering=False, debug=False)

    xh_d = nc.dram_tensor(
        "xh", [NB, NDMA, P, CGRP * tblk], bf16, kind="ExternalInput"
    ).ap()
    bpk_d = nc.dram_tensor("Bpk", [P, CH * R], bf16, kind="ExternalInput").ap()
    a2_d = nc.dram_tensor("A2", [R, F_out], bf16, kind="ExternalInput").ap()
    out_d = nc.dram_tensor("out", [T, F_out], bf16, kind="ExternalOutput").ap()

    with tile.TileContext(nc) as tc, ExitStack() as ctx:
        cpool = ctx.enter_context(tc.tile_pool(name="const", bufs=1))
        xtpool = ctx.enter_context(tc.tile_pool(name="xt", bufs=3 * NDMA))
        ypool = ctx.enter_context(tc.tile_pool(name="yt", bufs=2))
        opool = ctx.enter_context(tc.tile_pool(name="osb", bufs=8))
        y_pp = ctx.enter_context(tc.tile_pool(name="y_ps", bufs=2, space="PSUM"))
        o_pp = ctx.enter_context(tc.tile_pool(name="o_ps", bufs=3, space="PSUM"))

        bpk_sb = cpool.tile([P, CH * R], bf16, tag="bpk_sb")
        nc.sync.dma_start(bpk_sb[:], bpk_d)
        # A2 replicated at partition bases 0 and 32 for the row-tiled mm2.
        a2_sb = cpool.tile([RB + R, F_out], bf16, tag="a2_sb")
        nc.sync.dma_start(a2_sb[:R, :], a2_d)
        nc.sync.dma_start(a2_sb[RB:RB + R, :], a2_d)

        state = {}

        for blk in range(NB + 1):
            xts = []
            if blk < NB:
                for s in range(NDMA):
                    x_sb = xtpool.tile([P, CGRP, tblk], bf16, tag="x_sb")
                    eng = nc.sync if (s == 0 or blk % 2 == 1) else nc.gpsimd
                    eng.dma_start(
                        x_sb[:].rearrange("p c t -> p (c t)"), xh_d[blk, s]
                    )
                    xts.append(x_sb)
                ps_y = y_pp.tile([3 * RB + R, 512], f32, tag="ps_y")

            # All of mm2(blk-1) is emitted BEFORE mm1(blk): mm2 depends only
            # on y, never on fresh input, so it must not sit behind
            # input-waiting mm1 matmuls in the in-order PE queue (that
            # head-of-line blocking delayed the output stream by ~15us).
            if blk > 0:
                y_pk, _ = state.pop(blk - 1)
                o_sb0 = opool.tile(
                    [P, F_out], bf16, tag="o_sb0", name=f"o_sb0_{blk}"
                )
                o_sb1 = opool.tile(
                    [P, F_out], bf16, tag="o_sb1", name=f"o_sb1_{blk}"
                )
                trow = (blk - 1) * tblk
                for n in range(NS):
                    cs = slice(n * 512, (n + 1) * 512)
                    o_ps0 = o_pp.tile([P, 512], f32, tag="o_ps0")
                    o_ps1 = o_pp.tile([P, 512], f32, tag="o_ps1")
                    nc.tensor.matmul(
                        o_ps0[:], y_pk[:R, :], a2_sb[:R, cs],
                        start=True, stop=True,
                    )
                    nc.tensor.matmul(
                        o_ps1[:], y_pk[RB:RB + R, :], a2_sb[RB:RB + R, cs],
                        start=True, stop=True,
                    )
                    nc.scalar.copy(o_sb0[:, cs], o_ps0[:])
                    nc.vector.tensor_copy(o_sb1[:, cs], o_ps1[:])
                    # Fire the output DMAs per 2048-col half so the output
                    # stream starts as early as possible.
                    if n % (NS // 2) == NS // 2 - 1:
                        hs = slice((n + 1 - NS // 2) * 512, (n + 1) * 512)
                        nc.scalar.dma_start(out_d[trow:trow + P, hs], o_sb0[:, hs])
                        nc.gpsimd.dma_start(
                            out_d[trow + P:trow + 2 * P, hs], o_sb1[:, hs]
                        )
            if blk < NB:
                for r in range(CH // CT):
                    for j in range(CT):
                        c = CT * r + j
                        nc.tensor.matmul(
                            ps_y[j * RB:j * RB + R, :tblk],
                            bpk_sb[:, c * R:(c + 1) * R],
                            xts[c // CGRP][:, c % CGRP, :],
                            start=(r == 0), stop=(r == CH // CT - 1),
                            tile_position=(0, j * RB),
                        )
            if blk < NB:
                # y = sum of the 4 col-tile strips; two 128-token halves at
                # partition bases 0/32 (the mm2 row-tile weight layout).
                # DVE reads at most one PSUM operand per op -> stage in SBUF.
                yt = ypool.tile([R, tblk], f32, tag="yt")
                nc.vector.tensor_copy(yt[:], ps_y[:R, :tblk])
                for j in range(1, CT - 1):
                    nc.vector.tensor_add(
                        yt[:], yt[:], ps_y[j * RB:j * RB + R, :tblk]
                    )
                j = CT - 1
                y_pk = ypool.tile([RB + R, P], bf16, tag="y_pk")
                nc.vector.tensor_add(
                    y_pk[:R, :], yt[:, :P], ps_y[j * RB:j * RB + R, :P]
                )
                nc.vector.tensor_add(
                    y_pk[RB:RB + R, :], yt[:, P:tblk],
                    ps_y[j * RB:j * RB + R, P:tblk]
                )
                state[blk] = (y_pk, [])

    nc.compile()
    return nc


def _pack_inputs(x2d, A, B, T_shard, F_in, R):
    """Shard x on tokens (transposed, bf16); replicate B/A2 packs."""
    import ml_dtypes

    bf16 = ml_dtypes.bfloat16
    CH = F_in // P

    Bb = B.astype(np.float32).astype(bf16)
    bpk = np.ascontiguousarray(
        Bb.reshape(CH, P, R).transpose(1, 0, 2).reshape(P, CH * R)
    )
    a2 = np.ascontiguousarray((SCALING * A.astype(np.float32)).astype(bf16))

    T = T_shard
    tblk = min(TBLK, T)
    NB = T // tblk
    NDMA = 2
    CGRP = CH // NDMA

    def pack(m):
        a = m.reshape(NDMA, CGRP, P, NB, tblk)
        a = a.transpose(3, 0, 2, 1, 4)
        return np.ascontiguousarray(a.reshape(NB, NDMA, P, CGRP * tblk))

    n_shards = x2d.shape[0] // T_shard
    in_maps = []
    for c in range(n_shards):
        xt = np.ascontiguousarray(x2d[c * T_shard:(c + 1) * T_shard].T)
        in_maps.append({"xh": pack(xt.astype(bf16)), "Bpk": bpk, "A2": a2})
    return in_maps


def kernel(x, A, B):
    from concourse.bass_utils import run_bass_kernel_spmd

    x = np.asarray(x, dtype=np.float32)
    A = np.asarray(A, dtype=np.float32)
    B = np.asarray(B, dtype=np.float32)
    orig_shape = x.shape
    x2d = x.reshape(-1, F_IN)
    T_shard = x2d.shape[0] // N_CORES

    key = (T_shard, F_IN, F_OUT, RANK)
    if key not in _CACHE:
        _CACHE[key] = _build_nc(T_shard, F_IN, F_OUT, RANK)
    nc = _CACHE[key]

    in_maps = _pack_inputs(x2d, A, B, T_shard, F_IN, RANK)
    res = run_bass_kernel_spmd(nc, in_maps, core_ids=list(range(N_CORES)))
    out = np.concatenate(
        [np.asarray(r["out"], dtype=np.float32) for r in res.results], axis=0
    )
    return out.reshape(*orig_shape[:-1], F_OUT)


# revision 22
# speedup vs baseline: 1.1807x; 1.0563x over previous
"""Trainium2 Bass kernel for LoRALayer: out = 2.0 * (x @ B) @ A.

x: [4, 4096, 4096] f32; A: [8, 4096] f32; B: [4096, 8] f32.
Sharding: data-parallel on the 16384 tokens across 8 cores (2048 each);
A/B replicated. Host-side prep (part of sharding): each core's x-shard is
shipped transposed (contraction dim on SBUF partitions) as a single bf16
stream; B and 2*A likewise bf16. Output leaves the device as bf16 and the
host upconverts to f32 during the gather. Total HBM traffic is 32 MiB/core
and the kernel is fabric-bound (~435 GB/s/core SBUF AXI, ~216 GB/s per
DGE queue); bf16 rounding of x dominates the ~7e-3 absmax-rel error
(gate 2e-2).

Per core (T=2048), per 256-token block, PE work is packed with
tile_position concurrency so the (HAM-cold) PE never gates the streams:
  mm1 (4x col-tiled): feature chunk 4r+j accumulates into ps_y[32j:32j+8]
      (independent PE column groups, separate xbus streams) -> 8 rounds of
      4 concurrent 128x8x256 matmuls.
  y   = sum of 4 strips, split into two 128-token halves at partition
      bases 0/32 (bf16), matching A2 replicated at bases 0/32.
  mm2 (2x row-tiled): per 512-col chunk of A2, two concurrent rank-8
      matmuls (row groups 0/1) -> two PSUM banks; ACT copies subtile 0,
      DVE copies subtile 1 (PSUM->SBUF bf16).
DMA queue balance (each DGE queue drives 8 SDMA engines, ~216 GB/s):
  sync   = in0 + in1-on-odd-blocks   (12.6 MB, input only, no coupling)
  gpsimd = in1-on-even-blocks + out1 (12.6 MB)
  scalar = out0 only, emitted right after its own ACT copies (8.4 MB,
           waits trivially satisfied -> no head-of-line stalls)
"""

import numpy as np

P = 128
F_IN = 4096
F_OUT = 4096
RANK = 8
N_CORES = 8
SCALING = 2.0
TBLK = 256             # token block: 2 subtiles of 128 tokens

_CACHE = {}


def _build_nc(T, F_in, F_out, R):
    """Build the single-core Bass program for a T-token shard."""
    from contextlib import ExitStack

    import concourse.mybir as mybir
    import concourse.tile as tile
    from concourse import bacc

    f32 = mybir.dt.float32
    bf16 = mybir.dt.bfloat16
    tblk = min(TBLK, T)
    CH = F_in // P          # feature chunks (32)
    NB = T // tblk          # token blocks (8)
    NSUB = tblk // P        # 128-token subtiles per block (2)
    NS = F_out // 512       # output column chunks (8)
    NDMA = 2                # input sub-DMAs per block (1MB granularity)
    CGRP = CH // NDMA
    RB = 32                 # partition-base alignment for engine APs
    CT = 4                  # mm1 column-tiling width

    nc = bacc.Bacc("TRN2", target_bir_lowering=False, debug=False)

    xh_d = nc.dram_tensor(
        "xh", [NB, NDMA, P, CGRP * tblk], bf16, kind="ExternalInput"
    ).ap()
    bpk_d = nc.dram_tensor("Bpk", [P, CH * R], bf16, kind="ExternalInput").ap()
    a2_d = nc.dram_tensor("A2", [R, F_out], bf16, kind="ExternalInput").ap()
    out_d = nc.dram_tensor("out", [T, F_out], bf16, kind="ExternalOutput").ap()

    with tile.TileContext(nc) as tc, ExitStack() as ctx:
        cpool = ctx.enter_context(tc.tile_pool(name="const", bufs=1))
        xtpool = ctx.enter_context(tc.tile_pool(name="xt", bufs=3 * NDMA))
        ypool = ctx.enter_context(tc.tile_pool(name="yt", bufs=2))
        opool = ctx.enter_context(tc.tile_pool(name="osb", bufs=8))
        y_pp = ctx.enter_context(tc.tile_pool(name="y_ps", bufs=2, space="PSUM"))
        o_pp = ctx.enter_context(tc.tile_pool(name="o_ps", bufs=3, space="PSUM"))

        bpk_sb = cpool.tile([P, CH * R], bf16, tag="bpk_sb")
        nc.sync.dma_start(bpk_sb[:], bpk_d)
        # A2 replicated at partition bases 0 and 32 for the row-tiled mm2.
        a2_sb = cpool.tile([RB + R, F_out], bf16, tag="a2_sb")
        nc.sync.dma_start(a2_sb[:R, :], a2_d)
        nc.sync.dma_start(a2_sb[RB:RB + R, :], a2_d)

        state = {}

        for blk in range(NB + 1):
            xts = []
            if blk < NB:
                for s in range(NDMA):
                    x_sb = xtpool.tile([P, CGRP, tblk], bf16, tag="x_sb")
                    eng = nc.sync if (s == 0 or blk % 2 == 1) else nc.gpsimd
                    eng.dma_start(
                        x_sb[:].rearrange("p c t -> p (c t)"), xh_d[blk, s]
                    )
                    xts.append(x_sb)
                ps_y = y_pp.tile([3 * RB + R, 512], f32, tag="ps_y")

            # All of mm2(blk-1) is emitted BEFORE mm1(blk): mm2 depends only
            # on y, never on fresh input, so it must not sit behind
            # input-waiting mm1 matmuls in the in-order PE queue (that
            # head-of-line blocking delayed the output stream by ~15us).
            if blk > 0:
                y_pk, _ = state.pop(blk - 1)
                o_sb0 = opool.tile(
                    [P, F_out], bf16, tag="o_sb0", name=f"o_sb0_{blk}"
                )
                o_sb1 = opool.tile(
                    [P, F_out], bf16, tag="o_sb1", name=f"o_sb1_{blk}"
                )
                trow = (blk - 1) * tblk
                for n in range(NS):
                    cs = slice(n * 512, (n + 1) * 512)
                    o_ps0 = o_pp.tile([P, 512], f32, tag="o_ps0")
                    o_ps1 = o_pp.tile([P, 512], f32, tag="o_ps1")
                    nc.tensor.matmul(
                        o_ps0[:], y_pk[:R, :], a2_sb[:R, cs],
                        start=True, stop=True,
                    )
                    nc.tensor.matmul(
                        o_ps1[:], y_pk[RB:RB + R, :], a2_sb[RB:RB + R, cs],
                        start=True, stop=True,
                    )
                    nc.scalar.copy(o_sb0[:, cs], o_ps0[:])
                    nc.vector.tensor_copy(o_sb1[:, cs], o_ps1[:])
                    # Fire the output DMAs per 2048-col half so the output
                    # stream starts as early as possible.
                    if n % (NS // 2) == NS // 2 - 1:
                        hs = slice((n + 1 - NS // 2) * 512, (n + 1) * 512)
                        nc.gpsimd.dma_start(
                            out_d[trow + P:trow + 2 * P, hs], o_sb1[:, hs]
                        )
                        nc.scalar.dma_start(out_d[trow:trow + P, hs], o_sb0[:, hs])
            if blk < NB:
                for r in range(CH // CT):
                    for j in range(CT):
                        c = CT * r + j
                        nc.tensor.matmul(
                            ps_y[j * RB:j * RB + R, :tblk],
                            bpk_sb[:, c * R:(c + 1) * R],
                            xts[c // CGRP][:, c % CGRP, :],
                            start=(r == 0), stop=(r == CH // CT - 1),
                            tile_position=(0, j * RB),
                        )
            if blk < NB:
                # y = sum of the 4 col-tile strips; two 128-token halves at
                # partition bases 0/32 (the mm2 row-tile weight layout).
                # DVE reads at most one PSUM operand per op -> stage in SBUF.
                yt = ypool.tile([R, tblk], f32, tag="yt")
                nc.scalar.copy(yt[:], ps_y[:R, :tblk])
                for j in range(1, CT - 1):
                    nc.vector.tensor_add(
                        yt[:], yt[:], ps_y[j * RB:j * RB + R, :tblk]
                    )
                j = CT - 1
                y_pk = ypool.tile([RB + R, P], bf16, tag="y_pk")
                nc.vector.tensor_add(
                    y_pk[:R, :], yt[:, :P], ps_y[j * RB:j * RB + R, :P]
                )
                nc.vector.tensor_add(
                    y_pk[RB:RB + R, :], yt[:, P:tblk],
                    ps_y[j * RB:j * RB + R, P:tblk]
                )
                state[blk] = (y_pk, [])

    nc.compile()
    return nc


def _pack_inputs(x2d, A, B, T_shard, F_in, R):
    """Shard x on tokens (transposed, bf16); replicate B/A2 packs."""
    import ml_dtypes

    bf16 = ml_dtypes.bfloat16
    CH = F_in // P

    Bb = B.astype(np.float32).astype(bf16)
    bpk = np.ascontiguousarray(
        Bb.reshape(CH, P, R).transpose(1, 0, 2).reshape(P, CH * R)
    )
    a2 = np.ascontiguousarray((SCALING * A.astype(np.float32)).astype(bf16))

    T = T_shard
    tblk = min(TBLK, T)
    NB = T // tblk
    NDMA = 2
    CGRP = CH // NDMA

    def pack(m):
        a = m.reshape(NDMA, CGRP, P, NB, tblk)
        a = a.transpose(3, 0, 2, 1, 4)
        return np.ascontiguousarray(a.reshape(NB, NDMA, P, CGRP * tblk))

    n_shards = x2d.shape[0] // T_shard
    in_maps = []
    for c in range(n_shards):
        xt = np.ascontiguousarray(x2d[c * T_shard:(c + 1) * T_shard].T)
        in_maps.append({"xh": pack(xt.astype(bf16)), "Bpk": bpk, "A2": a2})
    return in_maps


def kernel(x, A, B):
    from concourse.bass_utils import run_bass_kernel_spmd

    x = np.asarray(x, dtype=np.float32)
    A = np.asarray(A, dtype=np.float32)
    B = np.asarray(B, dtype=np.float32)
    orig_shape = x.shape
    x2d = x.reshape(-1, F_IN)
    T_shard = x2d.shape[0] // N_CORES

    key = (T_shard, F_IN, F_OUT, RANK)
    if key not in _CACHE:
        _CACHE[key] = _build_nc(T_shard, F_IN, F_OUT, RANK)
    nc = _CACHE[key]

    in_maps = _pack_inputs(x2d, A, B, T_shard, F_IN, RANK)
    res = run_bass_kernel_spmd(nc, in_maps, core_ids=list(range(N_CORES)))
    out = np.concatenate(
        [np.asarray(r["out"], dtype=np.float32) for r in res.results], axis=0
    )
    return out.reshape(*orig_shape[:-1], F_OUT)
